# revision 38
# baseline (speedup 1.0000x reference)
"""GQA attention layer for Trainium2, tensor-parallel over kv-heads on 8 NeuronCores.

Problem: x:(1,2048,2048) f32, causal mask; q/k/v/o projections with
NUM_HEADS=32, NUM_KV_HEADS=8, HEAD_DIM=128, GROUP=4.

Sharding: core c owns kv-head c and its 4 query heads (columns 4c*128..(4c+4)*128
of wq, rows of wo). Each core computes a partial y_c = attnout_c @ wo_c; the host
sums the 8 partials (written bf16) and adds bo.

fp8 DoubleRow on the q/k path: the Q and K projections run as e4m3
MatmulPerfMode.DoubleRow matmuls — lhsT [128,2,128] x rhs [128,2,512]
contracts TWO 128-deep k-tiles per instruction at ~2x bf16 MAC rate
(HW-verified ~1.9x; 64-partition DR is 2x SLOWER, so scores stay bf16).
x and wq/wk are quantized host-side (x*SX, w*SW, folded out at the PSUM
drain). Numerics (validated in numpy, HW matches to 5 digits): softmax
damps q/k-path quantization ~4x vs the v/o paths, so QK-fp8 costs
1.53e-2 mean-rel error (gate 2e-2) while fp8 anywhere else (v, e, ao,
wo) costs 2-3e-2 PER STAGE and stays bf16. V proj reads a separate
bf16 copy of x.

DMA lessons (HW-measured): slices of a big DRAM tensor have per-
partition-strided sources the DGE cannot coalesce — the startup stream
(wq + x8 chunk 0) ships as per-pair/per-quarter CONTIGUOUS dram
tensors, halving early delivery time. Only sync/scalar HWDGE rings are
usable for bulk (SWDGE ~5x slower, +20us when given the late chunks).
DMA queue order is globally fragile: inserting one small transfer
mid-stream (wk between wq quarters) cost +34us reproducibly — measure
every reorder.

Chunk-0 pipeline: P_qk(0) -> A(0) scores/exp (all heads) -> P_v(0) ->
A(0) AV+finalize, deferring the xb0/wv DMA deadline past the startup
crunch (fp8 halved PE time per byte of input, so the early window is
delivery-bound).

Dataflow on each core (transposed layout, no transposes of the probability
matrix). Per-chunk pipeline P(c) -> A(c) -> Y(c) over 4 i-chunks of 512:
  P(c): qT/kT/vT projections for chunk c, drained on DVE via tensor_scalar
        (scale+bias) so they don't interrupt the ACT exp stream;
        v[j,d] via 4 PE transposes of vT.
  A(c): per head h: for j-tile b in 0..4c+3 (off = left columns of the
        i-chunk that are fully causally masked for this j-tile):
          sT[j,i] = matmul(lhsT=kT_tile, rhs=qT[:, off:])   (1 big MM)
          e = exp(sT) on ACT (1/sqrt(d) folded into qT bias); only the
              128-wide triangular block of diagonal j-tiles needs the
              strip-mask multiply (DVE, [128,128]).
          softmax denominator: COLSUM='ve': DVE accumulates eSum += e
              in-place; one ones.T @ eSum PE matmul per head.
              COLSUM='pe': per-head batch of ones.T @ e_b PE matmuls.
          avT[d,i] += v_b.T @ e  (PE, lagged behind exp by LAG tiles)
        finalize: cast sums to f16, broadcast with a k=1 PE matmul, then
        reciprocal on the full [128,CH] tile (all DVE lanes);
        aoT = avpsum * recip (DVE, bf16)
  Y(c): y[i,hid] += aoT_head_tile.T @ wo_head (4 head k-tiles), f32;
        drains go PSUM->SBUF on whichever of ACT/DVE is free (nc.any),
        each quarter DMA'd out right after its drain.

Hardware lessons baked in:
  - Back-to-back big bf16 MMs issue at ~216ns (LDWEIGHTS hidden by the PE
    reorder window); M=1 colsum matmuls inside the stream would break that
    hiding (~400ns extra each), so the denominator is accumulated off-PE.
  - DMA rings process their queue in order but share HBM bandwidth; each
    dma_start also costs ~600ns of issuing-engine sequencer time. Inputs are
    host-relayouted into a few large blocks: wq streams on the sync ring in
    parallel with x-chunk-0 on the scalar ring (eighths), later tensors
    queued behind them. Dependent DMAs (y out) must not sit at the head of a
    compute engine's FIFO queue, so they stay on sync until the exp stream
    is done. The gpsimd/SWDGE ring is slow - no inputs on it.
  - GpSimd cross-partition reduce and partition_broadcast are far too slow
    for the inner loop; cheap k=1/M=1 PE matmuls do broadcast/colsum.

Causality: for i-chunk c (512 wide) only j-tiles 0..4c+3 are computed, and
within the 4 diagonal j-tiles the fully-masked left 128*dd columns are skipped
everywhere (scores, exp, mask, eSum, AV).
"""

import math

import numpy as np
import ml_dtypes

HIDDEN = 2048
HEAD_DIM = 128
NUM_HEADS = 32
NUM_KV = 8
GROUP = NUM_HEADS // NUM_KV
S = 2048
NCORES = 8
CH = 512                      # i-chunk width
NCH = S // CH                 # 4 i-chunks
KT = HIDDEN // 128            # 16 contraction tiles over hidden
KP = KT // 2                  # 8 DoubleRow contraction pairs
NJT = S // 128                # 16 j-tiles
INV_SQRT_D = 1.0 / math.sqrt(HEAD_DIM)
# fp8 scales for the q/k projection path (folded out at the PSUM drain).
# x ~ N(0,1) scaled by SX keeps values normal in e4m3; weights
# uniform(+-1/sqrt(H)) scaled by SW avoid the subnormal range.
SX = 8.0
SW = 256.0

# Module-level knobs for test.py (the grading harness uses the defaults).
TRACE = False
LAST_EXEC_NS = None
LAST_RESULTS = None

# tuning knobs
LAG = 5                 # j-tiles between exp and the AV matmul consuming it
COLSUM = "ve"           # 've': DVE eSum accumulator; 'pe': batched PE matmuls

_PROG_CACHE = {}


def _build(mode):
    """mode: 'causal' (skip upper blocks, strip-mask diag), 'full' (all-ones
    mask), 'generic' (multiplicative bf16 mask tiles from HBM)."""
    import concourse.bacc as bacc
    import concourse.tile as tile
    import concourse.mybir as mybir
    from concourse.masks import make_identity

    f32 = mybir.dt.float32
    bf16 = mybir.dt.bfloat16
    f16 = mybir.dt.float16
    f8 = mybir.dt.float8e4
    DR = mybir.MatmulPerfMode.DoubleRow
    Ident = mybir.ActivationFunctionType.Identity
    Exp = mybir.ActivationFunctionType.Exp
    Add = mybir.AluOpType.add
    Mult = mybir.AluOpType.mult

    nc = bacc.Bacc(None, target_bir_lowering=False)

    # host-relayouted inputs: x as 4 chunk-column blocks, twice: fp8 (for the
    # q/k projections, DoubleRow) and bf16 (for the v projection). Weights
    # k-tile-major in the free dim, so each is one large DMA.
    x8_d = [nc.dram_tensor(f"x8c{c}", [128, KT, CH], f8, kind="ExternalInput")
            if c > 0 else None for c in range(NCH)]
    # chunk 0 of x8 and wq are delivered as separate contiguous DRAM tensors
    # (2 pairs + 3 quarters): a contiguous source lets the DGE coalesce
    # descriptors, where a strided slice of a big tensor cannot.
    x8p_d = [nc.dram_tensor(f"x8p{i}", [128, 2, CH], f8, kind="ExternalInput")
             for i in range(2)]
    x8q_d = [nc.dram_tensor(f"x8q{i}", [128, 4, CH], f8, kind="ExternalInput")
             for i in range(1, 4)]
    x_d = [nc.dram_tensor(f"xc{c}", [128, KT * CH], bf16, kind="ExternalInput")
           for c in range(NCH)]
    wqp_d = [nc.dram_tensor(f"wqp{i}", [128, 2, GROUP * HEAD_DIM], f8,
                            kind="ExternalInput") for i in range(2)]
    wqq_d = [nc.dram_tensor(f"wqq{i}", [128, 4, GROUP * HEAD_DIM], f8,
                            kind="ExternalInput") for i in range(1, 4)]
    wk_d = nc.dram_tensor("wk", [128, KT, HEAD_DIM], f8, kind="ExternalInput")
    wv_d = nc.dram_tensor("wv", [128, KT * HEAD_DIM], bf16, kind="ExternalInput")
    wo_d = nc.dram_tensor("wo", [128, GROUP * HIDDEN], bf16, kind="ExternalInput")
    bias_d = nc.dram_tensor("biasp", [128, 6], f32, kind="ExternalInput")
    if mode == "causal":
        ms_d = nc.dram_tensor("mstrip", [128, 896], bf16, kind="ExternalInput")
    if mode == "generic":
        mk_d = nc.dram_tensor("maskT", [S, S], bf16, kind="ExternalInput")
    y_d = nc.dram_tensor("y", [S, HIDDEN], bf16, kind="ExternalOutput")

    def nblocks(c):
        return 4 * c + 4 if mode == "causal" else NJT

    with tile.TileContext(nc) as tc:
        with (
            tc.tile_pool(name="consts", bufs=1) as consts,
            tc.tile_pool(name="xw", bufs=1) as xw,
            tc.tile_pool(name="proj", bufs=1) as proj,
            tc.tile_pool(name="epool",
                         bufs=(20 if COLSUM == "pe" else max(LAG + 5, 18))) as epool,
            tc.tile_pool(name="esp", bufs=2) as esp,
            tc.tile_pool(name="rpool", bufs=2) as rpool,
            tc.tile_pool(name="ypool", bufs=2) as ypool,
            tc.tile_pool(name="pp", bufs=3, space="PSUM") as pp,
            tc.tile_pool(name="spp", bufs=3, space="PSUM") as spp,
            tc.tile_pool(name="avp", bufs=2, space="PSUM") as avp,
        ):
            # ---- constants (DMAs issued after the first x/wq eighths so the
            # first matmul's inputs head the sync ring) ----
            bias_sb = consts.tile([128, 6], f32, tag="bias", name="bias_sb")
            if mode == "causal":
                mstrip = consts.tile([128, 896], bf16, tag="mstrip", name="mstrip")
            ident = consts.tile([128, 128], bf16, tag="ident", name="ident")
            make_identity(nc, ident)
            # all-ones stationary matrix: ones.T @ eSum gives the softmax
            # denominators already broadcast across all 128 partitions in ONE
            # full-size (LDW-hideable) matmul
            ones_mat = consts.tile([128, 128], bf16, tag="ones_mat", name="ones_mat")
            nc.vector.memset(ones_mat, 1.0)

            # ---- input loads: few large DMAs, interleaved so the first
            # projection matmuls can start early ----
            wq_sb = xw.tile([128, KT, GROUP * HEAD_DIM], f8, tag="wq", name="wq_sb")
            wk_sb = xw.tile([128, KT, HEAD_DIM], f8, tag="wk", name="wk_sb")
            wv_sb = xw.tile([128, KT * HEAD_DIM], bf16, tag="wv", name="wv_sb")
            wo_sb = xw.tile([128, GROUP * HIDDEN], bf16, tag="wo", name="wo_sb")
            x8_sb = [xw.tile([128, KT, CH], f8, tag=f"x8c{c}", name=f"x8c{c}")
                     for c in range(NCH)]
            x_sb = [xw.tile([128, KT * CH], bf16, tag=f"xc{c}", name=f"xc{c}")
                    for c in range(NCH)]
            # Two DMA rings (sync + scalar). Rings process their own queue in
            # order but share HBM bandwidth, so startup-critical wq (sync) and
            # fp8 x-chunk-0 (scalar ring) stream in parallel as k-tile pairs,
            # with later tensors queued behind them in phase-consumption
            # order: P(0) QK needs wq/x8c0/wk; P(0) V needs wv + bf16 x0;
            # later chunks stream behind. Dependent DMAs (y outputs) stay on
            # sync only. The gpsimd/SWDGE ring is measurably slower — only
            # mstrip rides it.
            # Input DMAs are spread over FOUR engine queues (sync, scalar,
            # vector, tensor) so issue overhead doesn't serialize the early
            # stream; input DMAs have no waits, so they issue immediately at
            # program start without blocking the owning engine's compute.
            # Each tensor goes as one large DMA (>=2KB/partition lines),
            # except the leading wq/x8 pair so the first DoubleRow matmul
            # starts as early as possible. Ordering per queue matches
            # phase-consumption order.
            nc.sync.dma_start(out=wq_sb[:, 0:2, :], in_=wqp_d[0][:, :, :])
            nc.scalar.dma_start(out=x8_sb[0][:, 0:2, :], in_=x8p_d[0][:, :, :])
            nc.sync.dma_start(out=bias_sb, in_=bias_d[:, :])
            # mstrip is small, has no consumers until A(0), and the slow
            # gpsimd/SWDGE ring is otherwise idle
            if mode == "causal":
                nc.gpsimd.dma_start(out=mstrip, in_=ms_d[:, :])
            # remaining startup stream: pair 1 then 4-ktile quarters, each
            # its own contiguous DRAM tensor (the DGE coalesces descriptors
            # for contiguous sources; strided slices of one big tensor
            # cannot), with fine-grained completion so Q-proj pair g only
            # waits on its own piece
            nc.sync.dma_start(out=wq_sb[:, 2:4, :], in_=wqp_d[1][:, :, :])
            nc.scalar.dma_start(out=x8_sb[0][:, 2:4, :], in_=x8p_d[1][:, :, :])
            for q4 in range(1, 4):
                nc.sync.dma_start(out=wq_sb[:, 4 * q4:4 * q4 + 4, :],
                                  in_=wqq_d[q4 - 1][:, :, :])
                nc.scalar.dma_start(out=x8_sb[0][:, 4 * q4:4 * q4 + 4, :],
                                    in_=x8q_d[q4 - 1][:, :, :])
            XH = KT * CH // 2
            # wk rides scalar right behind the x8c0 quarters: it lands just
            # before the K projection needs it (~t0+10us), without touching
            # the order-sensitive wq stream on sync
            nc.scalar.dma_start(out=wk_sb, in_=wk_d[:, :, :])
            nc.scalar.dma_start(out=wv_sb, in_=wv_d[:, :])
            nc.sync.dma_start(out=x_sb[0][:, 0:XH], in_=x_d[0][:, 0:XH])
            nc.scalar.dma_start(out=x_sb[0][:, XH:], in_=x_d[0][:, XH:])
            nc.sync.dma_start(out=x8_sb[1], in_=x8_d[1][:, :, :])
            nc.scalar.dma_start(out=x_sb[1][:, 0:XH], in_=x_d[1][:, 0:XH])
            nc.sync.dma_start(out=x_sb[1][:, XH:], in_=x_d[1][:, XH:])
            nc.scalar.dma_start(out=x8_sb[2], in_=x8_d[2][:, :, :])
            nc.sync.dma_start(out=wo_sb, in_=wo_d[:, :])
            nc.scalar.dma_start(out=x_sb[2][:, 0:XH], in_=x_d[2][:, 0:XH])
            nc.sync.dma_start(out=x_sb[2][:, XH:], in_=x_d[2][:, XH:])
            nc.scalar.dma_start(out=x8_sb[3], in_=x8_d[3][:, :, :])
            nc.sync.dma_start(out=x_sb[3][:, 0:XH], in_=x_d[3][:, 0:XH])
            nc.scalar.dma_start(out=x_sb[3][:, XH:], in_=x_d[3][:, XH:])

            def xs(kt, c):
                return x_sb[c][:, kt * CH:(kt + 1) * CH]

            def xs8(g, c):
                return x8_sb[c][:, 2 * g:2 * g + 2, :]

            qT = {}
            kT_c = []
            v_sb = []
            aoT = {}
            mask_sb = {}

            QSC = INV_SQRT_D / (SX * SW)
            KSC = 1.0 / (SX * SW)

            def phase_P(c):
                # Q projection for chunk c (4 heads, fp8 DoubleRow over k-tile
                # pairs), then K (fp8 DR), V (bf16), V-transposes.
                # For chunk 0 the x8/wq pairs are still streaming in, so
                # consume them in arrival order (pair outer) with all four
                # head accumulators live — the avp pool is idle this early.
                if c == 0:
                    psq = [pp.tile([128, CH], f32, tag="pp", name=f"psq{h}_0")
                           if h < 2 else
                           avp.tile([128, CH], f32, tag="av", name=f"psq{h}_0")
                           for h in range(GROUP)]
                    for g in range(KP):
                        for h in range(GROUP):
                            nc.tensor.matmul(
                                psq[h],
                                lhsT=wq_sb[:, 2 * g:2 * g + 2,
                                           h * 128:(h + 1) * 128],
                                rhs=xs8(g, c),
                                start=(g == 0), stop=(g == KP - 1),
                                perf_mode=DR)
                    for h in range(GROUP):
                        qt_t = proj.tile([128, CH], bf16, tag=f"q{h}_{c}",
                                         name=f"q{h}_{c}")
                        nc.vector.tensor_scalar(qt_t, psq[h], QSC,
                                                bias_sb[:, h:h + 1], Mult, Add)
                        qT[(h, c)] = qt_t
                else:
                    for h in range(GROUP):
                        ps = pp.tile([128, CH], f32, tag="pp", name=f"psq{h}_{c}")
                        for g in range(KP):
                            nc.tensor.matmul(
                                ps,
                                lhsT=wq_sb[:, 2 * g:2 * g + 2,
                                           h * 128:(h + 1) * 128],
                                rhs=xs8(g, c),
                                start=(g == 0), stop=(g == KP - 1),
                                perf_mode=DR)
                        qt_t = proj.tile([128, CH], bf16, tag=f"q{h}_{c}",
                                         name=f"q{h}_{c}")
                        # drain on DVE (tensor_scalar: ps*scale + bias) so the
                        # proj drains don't interrupt the ACT exp stream
                        nc.vector.tensor_scalar(qt_t, ps, QSC,
                                                bias_sb[:, h:h + 1], Mult, Add)
                        qT[(h, c)] = qt_t
                ps = pp.tile([128, CH], f32, tag="pp", name=f"psk{c}")
                for g in range(KP):
                    nc.tensor.matmul(ps, lhsT=wk_sb[:, 2 * g:2 * g + 2, :],
                                     rhs=xs8(g, c),
                                     start=(g == 0), stop=(g == KP - 1),
                                     perf_mode=DR)
                kt_t = proj.tile([128, CH], bf16, tag=f"kT{c}", name=f"kT{c}")
                nc.vector.tensor_scalar(kt_t, ps, KSC, bias_sb[:, 4:5],
                                        Mult, Add)
                kT_c.append(kt_t)

            def phase_P_v(c):
                ps = pp.tile([128, CH], f32, tag="pp", name=f"psv{c}")
                for kt in range(KT):
                    nc.tensor.matmul(ps, lhsT=wv_sb[:, kt * 128:(kt + 1) * 128],
                                     rhs=xs(kt, c),
                                     start=(kt == 0), stop=(kt == KT - 1))
                vt_t = proj.tile([128, CH], bf16, tag=f"vT{c}", name=f"vT{c}")
                nc.vector.tensor_scalar(vt_t, ps, bias_sb[:, 5:6], None, Add)
                for dd in range(4):
                    b = 4 * c + dd
                    tp = spp.tile([128, 128], bf16, tag="s", name=f"tp{b}")
                    nc.tensor.transpose(
                        tp, vt_t[:, dd * 128:(dd + 1) * 128], ident)
                    vt = proj.tile([128, 128], bf16, tag=f"v{b}", name=f"v{b}")
                    nc.vector.tensor_copy(vt, tp)
                    v_sb.append(vt)

            def phase_A(c, deferred=None, fillers=None):
                nb = nblocks(c)
                if mode == "generic":
                    for b in range(nb):
                        if b not in mask_sb:
                            mask_sb[b] = proj.tile([128, CH], bf16, tag=f"m{b}",
                                                   name=f"m{b}")
                        nc.sync.dma_start(
                            out=mask_sb[b],
                            in_=mk_d[b * 128:(b + 1) * 128, c * CH:(c + 1) * CH])

                def off_of(b):
                    if mode == "causal" and b >= 4 * c:
                        return 128 * (b - 4 * c)
                    return 0

                pending_fin = [None]
                for h in range(GROUP):
                    # in deferred mode the av psum tile is allocated at
                    # tail-emission time (avp has only 2 bufs; 4 heads'
                    # scores run before any AV)
                    avbox = [None if deferred is not None else
                             avp.tile([128, CH], f32, tag="av", name=f"av{h}_{c}")]
                    esum = esp.tile([128, CH], bf16, tag="es", name=f"es{h}_{c}")
                    e_tiles = {}
                    kept = []  # (b, off, e) for COLSUM='pe'
                    # for the chunk's last head the finalize runs immediately,
                    # so keep the last two e tiles out of the serial eSum
                    # chain and fold them into rb_ps directly: the reciprocal
                    # chain then waits on exp, not on the DVE add chain
                    late_e = []
                    n_late = 2 if (COLSUM == "ve" and h == GROUP - 1) else 0

                    def tail(b, nb=nb, avbox=avbox, h=h, c=c, e_tiles=e_tiles):
                        if avbox[0] is None:
                            avbox[0] = avp.tile([128, CH], f32, tag="av",
                                                name=f"av{h}_{c}")
                        off, e = e_tiles.pop(b)
                        nc.tensor.matmul(avbox[0][:, off:], lhsT=v_sb[b],
                                         rhs=e[:, off:],
                                         start=(b == 0), stop=(b == nb - 1),
                                         skip_group_check=True)

                    for b in range(nb):
                        if b == 3 and deferred is None:
                            # emit the previous head's finalize chain here so
                            # its serial latency hides under this head's
                            # score stream
                            if pending_fin[0] is not None:
                                pending_fin[0]()
                                pending_fin[0] = None
                            # narrow diagonal tiles make this stream exp-
                            # gated on ACT; a Y-group of the previous chunk
                            # is ready PE work that fills the wait
                            if fillers:
                                fillers.pop(0)()
                        off = off_of(b)
                        w = CH - off
                        sp_t = spp.tile([128, CH], f32, tag="s", name=f"s{h}_{c}_{b}")
                        nc.tensor.matmul(
                            sp_t[:, off:],
                            lhsT=kT_c[b // 4][:, (b % 4) * 128:(b % 4 + 1) * 128],
                            rhs=qT[(h, c)][:, off:], start=True, stop=True)
                        e = epool.tile([128, CH], bf16, tag="e", name=f"e{h}_{c}_{b}")
                        nc.scalar.activation(e[:, off:], sp_t[:, off:], Exp)
                        if mode == "causal" and b >= 4 * c:
                            # only the first 128 columns of the narrowed
                            # window are partially masked (the triangular
                            # block); everything right of it is fully valid
                            nc.vector.tensor_tensor(
                                e[:, off:off + 128], e[:, off:off + 128],
                                mstrip[:, 384:512], op=Mult)
                        elif mode == "generic":
                            nc.vector.tensor_tensor(e, e, mask_sb[b], op=Mult)
                        if COLSUM == "ve":
                            if b >= nb - n_late:
                                late_e.append((off, e))
                            elif b == 0:
                                nc.vector.tensor_copy(esum, e)
                            else:
                                nc.vector.tensor_tensor(
                                    esum[:, off:], esum[:, off:], e[:, off:], op=Add)
                        else:
                            kept.append((b, off, e))
                        e_tiles[b] = (off, e)
                        if deferred is None and b >= LAG:
                            tail(b - LAG)

                    def tails(nb=nb, tail=tail):
                        for b in range(max(nb - LAG, 0) if deferred is None
                                       else 0, nb):
                            tail(b)
                    if deferred is None:
                        tails()

                    def finalize(h=h, avbox=avbox, esum=esum, kept=kept, nb=nb,
                                 late_e=late_e):
                        av = avbox[0]
                        rb_ps = spp.tile([128, CH], f32, tag="s",
                                         name=f"rbp{h}_{c}")
                        if COLSUM == "ve":
                            nc.tensor.matmul(rb_ps, lhsT=ones_mat, rhs=esum,
                                             start=True, stop=not late_e,
                                             skip_group_check=True)
                            for i, (off, e) in enumerate(late_e):
                                nc.tensor.matmul(rb_ps[:, off:], lhsT=ones_mat,
                                                 rhs=e[:, off:], start=False,
                                                 stop=(i == len(late_e) - 1),
                                                 skip_group_check=True)
                        else:
                            # batched per-head colsum over the kept e tiles
                            for b, off, e in kept:
                                nc.tensor.matmul(rb_ps[:, off:], lhsT=ones_mat,
                                                 rhs=e[:, off:],
                                                 start=(b == 0),
                                                 stop=(b == nb - 1),
                                                 skip_group_check=True)
                        rb = rpool.tile([128, CH], f32, tag="rb",
                                        name=f"rb{h}_{c}")
                        nc.vector.reciprocal_approx_fast(rb, rb_ps)
                        ao = proj.tile([128, CH], bf16, tag=f"ao{h}_{c}",
                                       name=f"ao{h}_{c}")
                        nc.vector.tensor_tensor(ao, av, rb, op=Mult)
                        aoT[(h, c)] = ao

                    if deferred is not None:
                        deferred.append((tails, finalize))
                    else:
                        pending_fin[0] = finalize
                if pending_fin[0] is not None:
                    pending_fin[0]()

            ysb_t = {}
            y_done = set()

            def phase_Y_group(c, it, nh, filler=False):
                        if (c, it, nh) in y_done:
                            return
                        y_done.add((c, it, nh))
                        if (c, it) not in ysb_t:
                            ysb_t[(c, it)] = ypool.tile(
                                [128, HIDDEN], bf16, tag="y", name=f"y{c}_{it}")
                        ysb = ysb_t[(c, it)]
                        # the very last output group runs as two N=256 halves
                        # in SEPARATE psum tiles (different banks), so the
                        # second half's matmuls don't serialize behind the
                        # first half's drain on the critical tail
                        last = (c == NCH - 1 and it == CH // 128 - 1
                                and nh == NCH - 1)
                        if last:
                            halves = [(0, 256), (256, 256)]
                            yps = [spp.tile([128, 256], f32, tag="s",
                                            name=f"ypl{i}") for i in range(2)]
                        else:
                            halves = [(0, CH)]
                            yps = [pp.tile([128, CH], f32, tag="pp",
                                           name=f"yp{c}_{it}_{nh}")]
                        for (lo, wdt), yp in zip(halves, yps):
                            for h in range(GROUP):
                                nc.tensor.matmul(
                                    yp[:, 0:wdt],
                                    lhsT=aoT[(h, c)][:, it * 128:(it + 1) * 128],
                                    rhs=wo_sb[:, h * HIDDEN + nh * CH + lo:
                                              h * HIDDEN + nh * CH + lo + wdt],
                                    start=(h == 0), stop=(h == GROUP - 1),
                                    skip_group_check=last)
            # nc.any: pinning these to ACT was tried and
                            # regressed ~15us — a copy waiting on its yp
                            # matmul at the head of ACT's strict FIFO queue
                            # blocks the exp stream queued behind it.
                            # Exception: in the final i-tile the exp stream is
                            # long done, so alternating DVE/ACT explicitly
                            # halves the serial drain chain on the tail.
                            if c == NCH - 1 and it == CH // 128 - 1:
                                dst = ysb[:, nh * CH + lo:nh * CH + lo + wdt]
                                if nh % 2 == 0:
                                    nc.vector.tensor_copy(dst, yp[:, 0:wdt])
                                else:
                                    nc.scalar.copy(dst, yp[:, 0:wdt])
                            elif filler:
                                # filler groups run inside an exp-gated A
                                # stream: keep their drains off ACT
                                nc.vector.tensor_copy(
                                    ysb[:, nh * CH + lo:nh * CH + lo + wdt],
                                    yp[:, 0:wdt])
                            else:
                                nc.any.tensor_copy(
                                    ysb[:, nh * CH + lo:nh * CH + lo + wdt],
                                    yp[:, 0:wdt])
                            # block DMA right after its drain. Sync ring:
                            # dependent DMA issues must not sit on compute
                            # queues. Exception: by the last chunk the exp
                            # stream is done, so the scalar ring helps drain
                            # the final burst.
                            yeng = nc.scalar if c == NCH - 1 else nc.sync
                            yeng.dma_start(
                                out=y_d[c * CH + it * 128:
                                        c * CH + (it + 1) * 128,
                                        nh * CH + lo:nh * CH + lo + wdt],
                                in_=ysb[:, nh * CH + lo:nh * CH + lo + wdt])

            def phase_Y(c):
                for it in range(CH // 128):
                    for nh in range(NCH):
                        phase_Y_group(c, it, nh)

            # chunk 0: scores/exp for all heads run between the QK and V
            # projections, deferring the V-proj's xb0/wv DMA deadline past
            # the startup crunch; AV tails + finalizes follow V.
            phase_P(0)
            d0 = []
            phase_A(0, deferred=d0)
            phase_P_v(0)
            for t_fn, f_fn in d0:
                t_fn()
                f_fn()
            phase_P(1)
            phase_P_v(1)
            phase_A(1)
            phase_Y(0)
            phase_P(2)
            phase_P_v(2)
            # A(2)'s diagonal region is exp-gated too: feed it the first
            # Y(1) groups as PE fillers
            phase_A(2, fillers=[
                (lambda it=it, nh=nh: phase_Y_group(1, it, nh, filler=True))
                for it, nh in [(0, 0), (0, 1), (0, 2), (0, 3)]])
            phase_Y(1)
            phase_P(3)
            phase_P_v(3)
            phase_A(3, fillers=[
                (lambda it=it, nh=nh: phase_Y_group(2, it, nh, filler=True))
                for it, nh in [(0, 0), (0, 1), (0, 2), (0, 3)]])
            phase_Y(2)
            phase_Y(3)
    nc.finalize()
    return nc


def _get_prog(mode):
    if mode not in _PROG_CACHE:
        _PROG_CACHE[mode] = _build(mode)
    return _PROG_CACHE[mode]


def kernel(x, mask, wq, bq, wk, bk, wv, bv, wo, bo):
    global LAST_EXEC_NS, LAST_RESULTS
    from concourse.bass_utils import run_bass_kernel_spmd

    bf = ml_dtypes.bfloat16
    x = np.asarray(x, dtype=np.float32)
    mask = np.asarray(mask)
    wq = np.asarray(wq, dtype=np.float32)
    bq = np.asarray(bq, dtype=np.float32)
    wk = np.asarray(wk, dtype=np.float32)
    bk = np.asarray(bk, dtype=np.float32)
    wv = np.asarray(wv, dtype=np.float32)
    bv = np.asarray(bv, dtype=np.float32)
    wo = np.asarray(wo, dtype=np.float32)
    bo = np.asarray(bo, dtype=np.float32)

    m2 = mask[0, 0]
    if np.array_equal(m2 != 0, np.tril(np.ones((S, S), dtype=bool))):
        mode = "causal"
    elif np.all(m2 != 0):
        mode = "full"
    else:
        mode = "generic"

    f8 = ml_dtypes.float8_e4m3
    # x relayout: xc[c][p, kt*CH + j] = x[0][c*CH + j, kt*128 + p]
    xT = np.ascontiguousarray(x[0].T)                      # [H, S] f32
    xr = xT.reshape(KT, 128, NCH, CH).transpose(2, 1, 0, 3)  # [c, p, kt, j]
    xcs = [np.ascontiguousarray(xr[c].reshape(128, KT * CH)).astype(bf)
           for c in range(NCH)]
    # fp8 copy (scaled) for the q/k projection DoubleRow path
    x8s = [np.ascontiguousarray(xr[c] * SX).astype(f8).reshape(128, KT, CH)
           for c in range(NCH)]
    if mode == "causal":
        g = np.arange(896)[None, :]
        p = np.arange(128)[:, None]
        mstrip = (g - p >= 384).astype(bf)
    in_maps = []
    for core in range(NCORES):
        qs = slice(4 * core * 128, (4 * core + 4) * 128)
        ks = slice(core * 128, (core + 1) * 128)
        biasp = np.zeros((128, 6), np.float32)
        biasp[:, 0:4] = (bq[qs] * INV_SQRT_D).reshape(4, 128).T
        biasp[:, 4] = bk[ks]
        biasp[:, 5] = bv[ks]
        wq_r = np.ascontiguousarray(
            (wq[:, qs] * SW).astype(f8)
            .reshape(KT, 128, GROUP * HEAD_DIM).transpose(1, 0, 2))
        wk_r = np.ascontiguousarray(
            (wk[:, ks] * SW).astype(f8)
            .reshape(KT, 128, HEAD_DIM).transpose(1, 0, 2))
        wv_r = np.ascontiguousarray(
            wv[:, ks].astype(bf).reshape(KT, 128, HEAD_DIM).transpose(1, 0, 2)
            .reshape(128, KT * HEAD_DIM))
        wo_r = np.ascontiguousarray(
            wo[qs, :].astype(bf).reshape(GROUP, 128, HIDDEN).transpose(1, 0, 2)
            .reshape(128, GROUP * HIDDEN))
        im = {
            "wk": wk_r, "wv": wv_r, "wo": wo_r, "biasp": biasp,
            "wqp0": np.ascontiguousarray(wq_r[:, 0:2, :]),
            "wqp1": np.ascontiguousarray(wq_r[:, 2:4, :]),
        }
        for i in range(1, 4):
            im[f"wqq{i}"] = np.ascontiguousarray(wq_r[:, 4 * i:4 * i + 4, :])
        im["x8p0"] = np.ascontiguousarray(x8s[0][:, 0:2, :])
        im["x8p1"] = np.ascontiguousarray(x8s[0][:, 2:4, :])
        for i in range(1, 4):
            im[f"x8q{i}"] = np.ascontiguousarray(x8s[0][:, 4 * i:4 * i + 4, :])
        for c in range(NCH):
            im[f"xc{c}"] = xcs[c]
            if c > 0:
                im[f"x8c{c}"] = x8s[c]
        if mode == "causal":
            im["mstrip"] = mstrip
        if mode == "generic":
            im["maskT"] = np.ascontiguousarray((m2 != 0).T).astype(bf)
        in_maps.append(im)

    nc = _get_prog(mode)
    res = run_bass_kernel_spmd(nc, in_maps, list(range(NCORES)), trace=TRACE)
    LAST_EXEC_NS = res.exec_time_ns
    LAST_RESULTS = res
    y = np.zeros((S, HIDDEN), np.float64)
    for r in res.results:
        y += r["y"].astype(np.float64)
    y = (y + bo.astype(np.float64)).astype(np.float32)
    return y[None]

